# revision 9
# baseline (speedup 1.0000x reference)
"""Trainium2 Bass kernel for nn_Based_40630390620259 (sparse_attention).

Architecture ("Based"-style): linear (Taylor feature-map) attention +
windowed softmax attention, 16 heads, S=2048, D=1024.

Math identities used (verified against the reference to 1e-6):
  - Taylor feature map inner product collapses:
        qf.kf = 1 + (q.k)/4 + (q.k)^2/32 = 0.5 + 0.5*(1 + q.k/4)^2
    so the 273-dim feature space is never materialized. With Wq,Wk scaled
    by 0.5 on the host, the PE produces m' = q.k/4 (K=16 matmul) and the
    ACT engine computes Square(m' + 1) using its free bias.
  - The 0.5 factor is folded into the V projection weights; the +0.5
    constant term contributes a causal cumulative sum CUM of the
    (0.5-scaled) v rows, computed with 16 N=128 matmuls against an
    upper-triangular ones block + a recursive per-partition scalar-add.
    CUM row 64 (from the 0.5-constant column) equals 0.5*(s+1), which is
    exactly the constant-term part of the reference denominator.
  - win path: scores^T computed as [t,s] tiles; softmax denominator via a
    ones-column in V'; division deferred through the output projection via
    a K=1 outer-product broadcast of the reciprocal row.

Sharding: tensor-parallel over heads, 2 heads per core, 8 cores. Each core
produces a partial [S, D] output (its heads' contribution); the host sums.

Self-contained: only imports concourse/* from the environment.
"""

import numpy as np
import ml_dtypes

S = 2048
D = 1024
H = 16
FD = 16
HD = 64
W = 256
EPS = 1e-9
NCORES = 8

BF = ml_dtypes.bfloat16

_CACHE = {}


def _build_nc():
    import concourse.bass as bass
    import concourse.mybir as mybir
    import concourse.tile as tile
    from concourse import bacc
    from concourse.bass import ts

    f32 = mybir.dt.float32
    bf16 = mybir.dt.bfloat16
    MULT = mybir.AluOpType.mult
    ADD = mybir.AluOpType.add
    Exp = mybir.ActivationFunctionType.Exp
    Square = mybir.ActivationFunctionType.Square

    nc = bacc.Bacc("TRN2", target_bir_lowering=False)

    ht_d = nc.dram_tensor("ht", [D, S], bf16, kind="ExternalInput")
    wqk_d = nc.dram_tensor("wqk", [D, 384], bf16, kind="ExternalInput")
    wv_d = nc.dram_tensor("wv", [D, 256], bf16, kind="ExternalInput")
    wo_d = nc.dram_tensor("wo", [256, D], bf16, kind="ExternalInput")
    msk_d = nc.dram_tensor("msk", [128, 512], bf16, kind="ExternalInput")
    out_d = nc.dram_tensor("out", [S, D], f32, kind="ExternalOutput")

    NJ = 4          # number of 512-wide s blocks
    SB = 512        # s block width
    NT = 16         # number of 128-wide t chunks

    with tile.TileContext(nc) as tc:
        with (
            tc.tile_pool(name="sb", bufs=1) as sb,
            tc.tile_pool(name="sqp", bufs=4) as sqp,
            tc.tile_pool(name="exp", bufs=4) as exq,
            tc.tile_pool(name="stg", bufs=3) as stg,
            tc.tile_pool(name="psA", bufs=2, space="PSUM") as psA,
            tc.tile_pool(name="psAcc", bufs=3, space="PSUM") as psAcc,
            tc.tile_pool(name="psT", bufs=1, space="PSUM") as psT,
            tc.tile_pool(name="psO", bufs=2, space="PSUM") as psO,
        ):
            # ---------------- persistent SBUF tiles ----------------
            ht_sb = sb.tile([128, 8, S], bf16, name="ht_sb")
            wqk_sb = sb.tile([128, 8, 384], bf16, name="wqk_sb")
            wv_sb = sb.tile([128, 8, 256], bf16, name="wv_sb")
            wo_sb = sb.tile([128, 2, 1024], bf16, name="wo_sb")
            msk_sb = sb.tile([128, 512], bf16, name="msk_sb")
            ones_sb = sb.tile([128, 64], bf16, name="ones_sb")
            qw_sb = sb.tile([128, S], bf16, name="qw_sb")
            kw_sb = sb.tile([128, S], bf16, name="kw_sb")
            qg_sb = [sb.tile([16, S], bf16, name=f"qg{h}_sb") for h in range(2)]
            kg_sb = [sb.tile([16, S], bf16, name=f"kg{h}_sb") for h in range(2)]
            v4_sb = sb.tile([128, NT, 260], bf16, name="v4_sb")
            cum_sb = [sb.tile([65, S], f32, name=f"cum{h}_sb") for h in range(2)]
            ul_sb = [sb.tile([65, S], f32, name=f"ul{h}_sb") for h in range(2)]
            uw_sb = [sb.tile([65, S], f32, name=f"uw{h}_sb") for h in range(2)]
            scl_sb = [sb.tile([128, S], bf16, name=f"scl{h}_sb") for h in range(2)]
            den_sb = sb.tile([4, S], f32, name="den_sb")
            recf_sb = sb.tile([4, S], f32, name="recf_sb")
            recb_sb = sb.tile([4, S], bf16, name="recb_sb")

            # ---------------- load inputs ----------------
            for k in range(8):
                nc.sync.dma_start(out=ht_sb[:, k, :], in_=ht_d[ts(k, 128), :])
                nc.sync.dma_start(out=wqk_sb[:, k, :], in_=wqk_d[ts(k, 128), :])
                nc.sync.dma_start(out=wv_sb[:, k, :], in_=wv_d[ts(k, 128), :])
            for k in range(2):
                nc.sync.dma_start(out=wo_sb[:, k, :], in_=wo_d[ts(k, 128), :])
            nc.sync.dma_start(out=msk_sb[:, :], in_=msk_d[:, :])
            nc.gpsimd.memset(ones_sb[:, :], 1.0)
            # constant columns of v4: 0.5 for lin heads, 1.0 for win heads
            v4r = v4_sb.rearrange("p s (g x) -> p s g x", x=65)
            nc.gpsimd.memset(v4r[:, :, 0:2, 64], 0.5)
            nc.gpsimd.memset(v4r[:, :, 2:4, 64], 1.0)

            # ---------------- phase 1a: q/k projections ----------------
            # wqk columns: [qw_a(64) qw_b(64) | kw_a(64) kw_b(64) |
            #               qlin_a(16) qlin_b(16) klin_a(16) klin_b(16)]
            for blk in range(3):
                m = 128
                c0 = blk * 128
                for j in range(NJ):
                    pp = psA.tile([128, SB], f32, name="pp", tag="psA")
                    for k in range(8):
                        nc.tensor.matmul(
                            pp[0:m, :],
                            lhsT=wqk_sb[:, k, c0 : c0 + m],
                            rhs=ht_sb[:, k, ts(j, SB)],
                            start=(k == 0),
                            stop=(k == 7),
                        )
                    js = ts(j, SB)
                    if blk == 0:
                        nc.vector.tensor_copy(out=qw_sb[:, js], in_=pp[:, :])
                    elif blk == 1:
                        nc.vector.tensor_copy(out=kw_sb[:, js], in_=pp[:, :])
                    else:
                        nc.vector.tensor_copy(out=qg_sb[0][:, js], in_=pp[0:16, :])
                        nc.vector.tensor_copy(out=qg_sb[1][:, js], in_=pp[32:48, :])
                        nc.vector.tensor_copy(out=kg_sb[0][:, js], in_=pp[64:80, :])
                        nc.vector.tensor_copy(out=kg_sb[1][:, js], in_=pp[96:112, :])

            # ---------------- phase 1b: v projections ----------------
            # wv columns: [vlin_a vlin_b vwin_a vwin_b] (lin ones pre-scaled 0.5)
            for st in range(NT):
                pv = psA.tile([128, SB], f32, name="pv", tag="psA")
                for k in range(8):
                    nc.tensor.matmul(
                        pv[:, 0:256],
                        lhsT=ht_sb[:, k, ts(st, 128)],
                        rhs=wv_sb[:, k, :],
                        start=(k == 0),
                        stop=(k == 7),
                    )
                nc.vector.tensor_copy(
                    out=v4r[:, st, :, 0:64],
                    in_=pv[:, 0:256].rearrange("p (g x) -> p g x", x=64),
                )

            # ---------------- phase 1c: CUM (cumulative v-half sums) -------
            # cum[h][d, s] = sum_{t<=s} vlin_half[t, d]; row 64 = 0.5*(s+1)
            for h in range(2):
                lin_sl = slice(65 * h, 65 * h + 65)
                for sj in range(NT):
                    icp = psA.tile([128, SB], f32, name="icp", tag="psA")
                    nc.tensor.matmul(
                        icp[0:65, 0:128],
                        lhsT=v4_sb[:, sj, lin_sl],
                        rhs=msk_sb[:, 0:128],
                        start=True,
                        stop=True,
                    )
                    if sj == 0:
                        nc.vector.tensor_scalar(
                            out=cum_sb[h][:, 0:128],
                            in0=icp[0:65, 0:128],
                            scalar1=0.0,
                            scalar2=None,
                            op0=ADD,
                        )
                    else:
                        nc.vector.tensor_scalar(
                            out=cum_sb[h][:, ts(sj, 128)],
                            in0=icp[0:65, 0:128],
                            scalar1=cum_sb[h][:, sj * 128 - 1 : sj * 128],
                            scalar2=None,
                            op0=ADD,
                        )

            # ---------------- phase 2 + pipelined tails ----------------
            def phase2(j):
                js = ts(j, SB)
                for h in range(2):
                    # ----- linear-attention chain -----
                    lim_l = 4 * j + 4
                    qkv = psAcc.tile([65, SB], f32, name="qkv", tag="acc")
                    sq_tiles = {}
                    for ti in range(lim_l):
                        mp = psA.tile([128, SB], f32, name="mp", tag="psA")
                        nc.tensor.matmul(
                            mp[:, :],
                            lhsT=kg_sb[h][:, ts(ti, 128)],
                            rhs=qg_sb[h][:, js],
                            start=True,
                            stop=True,
                        )
                        sq = sqp.tile([128, SB], bf16, name="sq", tag="sq")
                        col0 = max(0, ti - 4 * j) * 128
                        if col0:
                            nc.gpsimd.memset(sq[:, 0:col0], 0.0)
                        # sq = (m' + 1)^2, the +1 via ACT's free bias
                        nc.scalar.activation(
                            sq[:, col0:SB], mp[:, col0:SB], Square, bias=1.0
                        )
                        sd = ti - 4 * j
                        if 0 <= sd <= 3:
                            dsl = ts(sd, 128)
                            nc.gpsimd.tensor_tensor(
                                out=sq[:, dsl], in0=sq[:, dsl],
                                in1=msk_sb[:, 0:128], op=MULT,
                            )
                        sq_tiles[ti] = sq
                        # pipelined accumulation (one step behind)
                        if ti >= 1:
                            nc.tensor.matmul(
                                qkv[:, :],
                                lhsT=v4_sb[:, ti - 1, slice(65 * h, 65 * h + 65)],
                                rhs=sq_tiles[ti - 1][:, :],
                                start=(ti - 1 == 0),
                                stop=False,
                                skip_group_check=True,
                            )
                    nc.tensor.matmul(
                        qkv[:, :],
                        lhsT=v4_sb[:, lim_l - 1, slice(65 * h, 65 * h + 65)],
                        rhs=sq_tiles[lim_l - 1][:, :],
                        start=(lim_l - 1 == 0),
                        stop=True,
                        skip_group_check=True,
                    )
                    # unscaled lin numerator (+CUM); row 64 = full denominator
                    nc.vector.tensor_tensor(
                        out=ul_sb[h][:, js],
                        in0=qkv[0:65, :],
                        in1=cum_sb[h][:, js],
                        op=ADD,
                    )
                    nc.sync.dma_start(
                        out=den_sb[h : h + 1, js],
                        in_=ul_sb[h][64:65, js],
                    )
                    # ----- windowed-attention chain -----
                    lim_w = min(16, 4 * j + 6)
                    hsl = slice(64 * h, 64 * h + 64)
                    nt = psAcc.tile([65, SB], f32, name="nt", tag="acc")
                    ex_tiles = {}
                    for ti in range(lim_w):
                        sp = psA.tile([128, SB], f32, name="sp", tag="psA")
                        nc.tensor.matmul(
                            sp[:, :],
                            lhsT=kw_sb[hsl, ts(ti, 128)],
                            rhs=qw_sb[hsl, js],
                            start=True,
                            stop=True,
                        )
                        ex = exq.tile([128, SB], bf16, name="ex", tag="ex")
                        col0 = max(0, ti - 2 - 4 * j) * 128
                        if col0:
                            nc.gpsimd.memset(ex[:, 0:col0], 0.0)
                        nc.scalar.activation(
                            ex[:, col0:SB], sp[:, col0:SB], Exp, scale=0.125
                        )
                        sd = ti - 2 - 4 * j
                        if 0 <= sd <= 3:
                            dsl = ts(sd, 128)
                            nc.gpsimd.tensor_tensor(
                                out=ex[:, dsl], in0=ex[:, dsl],
                                in1=msk_sb[:, 128:256], op=MULT,
                            )
                        ex_tiles[ti] = ex
                        if ti >= 1:
                            nc.tensor.matmul(
                                nt[:, :],
                                lhsT=v4_sb[:, ti - 1, slice(130 + 65 * h, 195 + 65 * h)],
                                rhs=ex_tiles[ti - 1][:, :],
                                start=(ti - 1 == 0),
                                stop=False,
                                skip_group_check=True,
                            )
                    nc.tensor.matmul(
                        nt[:, :],
                        lhsT=v4_sb[:, lim_w - 1, slice(130 + 65 * h, 195 + 65 * h)],
                        rhs=ex_tiles[lim_w - 1][:, :],
                        start=(lim_w - 1 == 0),
                        stop=True,
                        skip_group_check=True,
                    )
                    nc.vector.tensor_copy(out=uw_sb[h][:, js], in_=nt[0:65, :])
                    nc.sync.dma_start(
                        out=den_sb[2 + h : 3 + h, js],
                        in_=uw_sb[h][64:65, js],
                    )

            def tail(j):
                js = ts(j, SB)
                # reciprocal of eps-shifted denominators (partitions 0/32/64/96)
                nc.vector.tensor_scalar(
                    out=recf_sb[:, js], in0=den_sb[:, js],
                    scalar1=EPS, scalar2=None, op0=ADD,
                )
                nc.vector.reciprocal(out=recf_sb[:, js], in_=recf_sb[:, js])
                nc.gpsimd.tensor_copy(out=recb_sb[:, js], in_=recf_sb[:, js])
                for h in range(2):
                    for p in range(2):  # 0 = lin, 1 = win
                        r = 2 * p + h
                        u = ul_sb[h] if p == 0 else uw_sb[h]
                        bc = psT.tile([64, SB], f32, name="bc", tag="bc")
                        nc.tensor.matmul(
                            bc[:, :],
                            lhsT=msk_sb[0:4, 256 + 64 * r : 320 + 64 * r],
                            rhs=recb_sb[:, js],
                            start=True,
                            stop=True,
                        )
                        nc.vector.tensor_tensor(
                            out=scl_sb[h][ts(p, 64), js],
                            in0=u[0:64, js],
                            in1=bc[0:64, :],
                            op=MULT,
                        )
                for st in range(4 * j, 4 * j + 4):
                    for nb in range(2):
                        po = psO.tile([128, SB], f32, name="po", tag="po")
                        for h in range(2):
                            nc.tensor.matmul(
                                po[:, :],
                                lhsT=scl_sb[h][:, ts(st, 128)],
                                rhs=wo_sb[:, h, ts(nb, SB)],
                                start=(h == 0),
                                stop=(h == 1),
                                skip_group_check=True,
                            )
                        so = stg.tile([128, SB], f32, name="so", tag="so")
                        nc.vector.tensor_copy(out=so[:, :], in_=po[:, :])
                        nc.sync.dma_start(
                            out=out_d[ts(st, 128), ts(nb, SB)], in_=so[:, :]
                        )

            phase2(0)
            phase2(1)
            tail(0)
            phase2(2)
            tail(1)
            phase2(3)
            tail(2)
            tail(3)

    nc.compile()
    return nc


def _prep_inputs(inputs):
    """Host-side sharding/packing. Returns per-core input maps."""
    h = np.asarray(inputs["hidden_states"], np.float32).reshape(S, D)
    ht = np.ascontiguousarray(h.T).astype(BF)

    lin_Wq = np.asarray(inputs["lin_Wq"], np.float32)
    lin_Wk = np.asarray(inputs["lin_Wk"], np.float32)
    lin_Wv = np.asarray(inputs["lin_Wv"], np.float32)
    lin_Wo = np.asarray(inputs["lin_Wo"], np.float32)
    win_Wq = np.asarray(inputs["win_Wq"], np.float32)
    win_Wk = np.asarray(inputs["win_Wk"], np.float32)
    win_Wv = np.asarray(inputs["win_Wv"], np.float32)
    win_Wo = np.asarray(inputs["win_Wo"], np.float32)

    # constant mask tiles
    p = np.arange(128)[:, None]
    f = np.arange(128)[None, :]
    msk = np.zeros((128, 512), np.float32)
    msk[:, 0:128] = (p <= f)          # lin diag mask (t <= s)
    msk[:, 128:256] = (p < f)         # win partial mask (t < s)
    for r in range(4):                # row-selector for reciprocal broadcast
        msk[r, 256 + 64 * r : 320 + 64 * r] = 1.0

    in_maps = []
    for c in range(NCORES):
        a, b = 2 * c, 2 * c + 1
        wqk = np.zeros((D, 384), np.float32)
        wqk[:, 0:64] = win_Wq[:, a * HD : (a + 1) * HD]
        wqk[:, 64:128] = win_Wq[:, b * HD : (b + 1) * HD]
        wqk[:, 128:192] = win_Wk[:, a * HD : (a + 1) * HD]
        wqk[:, 192:256] = win_Wk[:, b * HD : (b + 1) * HD]
        wqk[:, 256:272] = lin_Wq[:, a * FD : (a + 1) * FD] * 0.5
        wqk[:, 288:304] = lin_Wq[:, b * FD : (b + 1) * FD] * 0.5
        wqk[:, 320:336] = lin_Wk[:, a * FD : (a + 1) * FD] * 0.5
        wqk[:, 352:368] = lin_Wk[:, b * FD : (b + 1) * FD] * 0.5
        wv = np.zeros((D, 256), np.float32)
        wv[:, 0:64] = lin_Wv[:, a * HD : (a + 1) * HD] * 0.5
        wv[:, 64:128] = lin_Wv[:, b * HD : (b + 1) * HD] * 0.5
        wv[:, 128:192] = win_Wv[:, a * HD : (a + 1) * HD]
        wv[:, 192:256] = win_Wv[:, b * HD : (b + 1) * HD]
        wo = np.zeros((256, D), np.float32)
        wo[0:64] = lin_Wo[a * HD : (a + 1) * HD]
        wo[64:128] = win_Wo[a * HD : (a + 1) * HD]
        wo[128:192] = lin_Wo[b * HD : (b + 1) * HD]
        wo[192:256] = win_Wo[b * HD : (b + 1) * HD]
        in_maps.append(
            {
                "ht": ht,
                "wqk": wqk.astype(BF),
                "wv": wv.astype(BF),
                "wo": wo.astype(BF),
                "msk": msk.astype(BF),
            }
        )
    return in_maps


def kernel(**inputs) -> np.ndarray:
    from concourse.bass_utils import run_bass_kernel_spmd

    if "nc" not in _CACHE:
        _CACHE["nc"] = _build_nc()
    nc = _CACHE["nc"]
    in_maps = _prep_inputs(inputs)
    res = run_bass_kernel_spmd(nc, in_maps, core_ids=list(range(NCORES)))
    out = np.zeros((S, D), np.float32)
    for r in res.results:
        out += r["out"]
    return out.reshape(1, S, D)


if __name__ == "__main__":
    nc = _build_nc()
    print("built ok")


# revision 19
# speedup vs baseline: 1.0181x; 1.0181x over previous
"""Trainium2 Bass kernel for nn_Based_40630390620259 (sparse_attention).

Architecture ("Based"-style): linear (Taylor feature-map) attention +
windowed softmax attention, 16 heads, S=2048, D=1024.

Math identities used (verified against the reference to 1e-6):
  - Taylor feature map inner product collapses:
        qf.kf = 1 + (q.k)/4 + (q.k)^2/32 = 0.5 + 0.5*(1 + q.k/4)^2
    so the 273-dim feature space is never materialized. With Wq,Wk scaled
    by 0.5 on the host and a constant ones-row appended to q/k (K=17
    matmul), the PE produces m'' = 1 + q.k/4 directly; sq = m''^2 on DVE.
  - The 0.5 factor is folded into the V projection weights; the +0.5
    constant term contributes a causal cumulative sum CUM of the
    (0.5-scaled) v rows, computed with 16 N=128 matmuls against an
    upper-triangular ones block + a recursive per-partition scalar-add.
    CUM row 64 (from the 0.5-constant column) equals 0.5*(s+1), which is
    exactly the constant-term part of the reference denominator.
  - win path: scores^T computed as [t,s] tiles; softmax denominator via a
    ones-column in V'; division deferred through the output projection via
    a gpsimd partition_broadcast of the reciprocal row.

Sharding: tensor-parallel over heads, 2 heads per core, 8 cores. Each core
produces a partial [S, D] output (its heads' contribution); the host sums.
Both heads are processed per t-chunk with 2-way row-strip packing (lin at
array rows 0/32, win at rows 0/64) writing the two halves of paired
[128,1024] PSUM tiles, so elementwise ops cover both heads in one
instruction.

Self-contained: only imports concourse/* from the environment.
"""

import numpy as np
import ml_dtypes

S = 2048
D = 1024
H = 16
FD = 16
HD = 64
W = 256
EPS = 1e-9
NCORES = 8

BF = ml_dtypes.bfloat16

_CACHE = {}


def _build_nc(dbg=False):
    import concourse.bass as bass
    import concourse.mybir as mybir
    import concourse.tile as tile
    from concourse import bacc
    from concourse.bass import ts

    f32 = mybir.dt.float32
    bf16 = mybir.dt.bfloat16
    MULT = mybir.AluOpType.mult
    ADD = mybir.AluOpType.add
    Exp = mybir.ActivationFunctionType.Exp
    Square = mybir.ActivationFunctionType.Square

    nc = bacc.Bacc("TRN2", target_bir_lowering=False)

    ht_d = nc.dram_tensor("ht", [D, S], bf16, kind="ExternalInput")
    wqk_d = nc.dram_tensor("wqk", [D, 384], bf16, kind="ExternalInput")
    wv_d = nc.dram_tensor("wv", [D, 256], bf16, kind="ExternalInput")
    wo_d = nc.dram_tensor("wo", [256, D], bf16, kind="ExternalInput")
    msk_d = nc.dram_tensor("msk", [128, 256], bf16, kind="ExternalInput")
    orow_d = nc.dram_tensor("orow", [1, S], bf16, kind="ExternalInput")
    out_d = nc.dram_tensor("out", [S, D], f32, kind="ExternalOutput")
    if dbg:
        dbg_t = {
            "d_qw": nc.dram_tensor("d_qw", [128, S], f32, kind="ExternalOutput"),
            "d_kw": nc.dram_tensor("d_kw", [128, S], f32, kind="ExternalOutput"),
            "d_qkg": nc.dram_tensor("d_qkg", [128, 2 * S], f32, kind="ExternalOutput"),
            "d_v4": nc.dram_tensor("d_v4", [128, 16 * 260], f32, kind="ExternalOutput"),
            "d_cum0": nc.dram_tensor("d_cum0", [65, S], f32, kind="ExternalOutput"),
            "d_ul0": nc.dram_tensor("d_ul0", [65, S], f32, kind="ExternalOutput"),
            "d_uw0": nc.dram_tensor("d_uw0", [65, S], f32, kind="ExternalOutput"),
            "d_den": nc.dram_tensor("d_den", [128, S], f32, kind="ExternalOutput"),
            "d_recf": nc.dram_tensor("d_recf", [128, S], f32, kind="ExternalOutput"),
            "d_scl0": nc.dram_tensor("d_scl0", [128, S], f32, kind="ExternalOutput"),
        }

    NJ = 4          # number of 512-wide s blocks
    SB = 512        # s block width
    NT = 16         # number of 128-wide t chunks

    with tile.TileContext(nc) as tc:
        with (
            tc.tile_pool(name="sb", bufs=1) as sb,
            tc.tile_pool(name="sqp", bufs=4) as sqp,
            tc.tile_pool(name="exp", bufs=4) as exq,
            tc.tile_pool(name="stg", bufs=3) as stg,
            tc.tile_pool(name="bct", bufs=2) as bct,
            tc.tile_pool(name="psA", bufs=2, space="PSUM") as psA,
            tc.tile_pool(name="psAcc", bufs=2, space="PSUM") as psAcc,
            tc.tile_pool(name="psO", bufs=1, space="PSUM") as psO,
        ):
            # ---------------- persistent SBUF tiles ----------------
            ht_sb = sb.tile([128, 8, S], bf16, name="ht_sb")
            wqk_sb = sb.tile([128, 8, 384], bf16, name="wqk_sb")
            wv_sb = sb.tile([128, 8, 256], bf16, name="wv_sb")
            wo_sb = sb.tile([128, 2, 1024], bf16, name="wo_sb")
            msk_sb = sb.tile([128, 256], bf16, name="msk_sb")
            # qkg: lin q/k with ones row; h0 rows 0:17, h1 rows 32:49;
            # free index 0 = q, 1 = k
            qkg_sb = sb.tile([128, 2, S], bf16, name="qkg_sb")
            qw_sb = sb.tile([128, S], bf16, name="qw_sb")
            kw_sb = sb.tile([128, S], bf16, name="kw_sb")
            v4_sb = sb.tile([128, NT, 260], bf16, name="v4_sb")
            cum_sb = [sb.tile([65, S], f32, name=f"cum{h}_sb") for h in range(2)]
            ul_sb = [sb.tile([65, S], f32, name=f"ul{h}_sb") for h in range(2)]
            uw_sb = [sb.tile([65, S], f32, name=f"uw{h}_sb") for h in range(2)]
            scl_sb = [sb.tile([128, S], bf16, name=f"scl{h}_sb") for h in range(2)]
            den_sb = sb.tile([128, S], f32, name="den_sb")
            recf_sb = sb.tile([128, S], f32, name="recf_sb")
            recr_sb = [sb.tile([1, S], f32, name=f"recr{r}_sb") for r in range(4)]

            v4r = v4_sb.rearrange("p s (g x) -> p s g x", x=65)

            # ---------------- load inputs ----------------
            for k in range(8):
                nc.sync.dma_start(out=ht_sb[:, k, :], in_=ht_d[ts(k, 128), :])
                nc.sync.dma_start(out=wqk_sb[:, k, :], in_=wqk_d[ts(k, 128), :])
                nc.sync.dma_start(out=wv_sb[:, k, :], in_=wv_d[ts(k, 128), :])
            for k in range(2):
                nc.sync.dma_start(out=wo_sb[:, k, :], in_=wo_d[ts(k, 128), :])
            nc.sync.dma_start(out=msk_sb[:, :], in_=msk_d[:, :])
            # constant columns of v4: 0.5 for lin heads, 1.0 for win heads
            nc.gpsimd.memset(v4r[:, :, 0:2, 64], 0.5)
            nc.gpsimd.memset(v4r[:, :, 2:4, 64], 1.0)
            # garbage rows of den must stay finite for the reciprocal pass
            nc.gpsimd.memset(den_sb[:, :], 1.0)

            # ---------------- phase 1a: q/k projections (paired j) ----------
            # wqk columns: [qw_a(64) qw_b(64) | kw_a(64) kw_b(64) |
            #   qlin_a@256 qlin_b@288 klin_a@320 klin_b@352 (16 each)]
            for blk in range(3):
                c0 = blk * 128
                for jp in range(2):
                    js2 = ts(jp, 1024)
                    pp = psA.tile([128, 1024], f32, name="pp", tag="psA")
                    for jh in range(2):
                        for k in range(8):
                            nc.tensor.matmul(
                                pp[:, ts(jh, SB)],
                                lhsT=wqk_sb[:, k, c0 : c0 + 128],
                                rhs=ht_sb[:, k, ts(2 * jp + jh, SB)],
                                start=(k == 0),
                                stop=(k == 7),
                            )
                    if blk == 0:
                        nc.vector.tensor_copy(out=qw_sb[:, js2], in_=pp[:, :])
                    elif blk == 1:
                        nc.vector.tensor_copy(out=kw_sb[:, js2], in_=pp[:, :])
                    else:
                        nc.vector.tensor_copy(
                            out=qkg_sb[0:48, 0, js2], in_=pp[0:48, :]
                        )
                        nc.vector.tensor_copy(
                            out=qkg_sb[0:48, 1, js2], in_=pp[64:112, :]
                        )
                        # restore ones rows clobbered by the 48-row copies
                        for qk in range(2):
                            for r in (16, 48):
                                nc.sync.dma_start(
                                    out=qkg_sb[r : r + 1, qk, js2],
                                    in_=orow_d[0:1, js2],
                                )

            # ---------------- phase 1b: v projections (paired st) -----------
            for sp_ in range(8):
                st0 = 2 * sp_
                pv = psA.tile([128, 1024], f32, name="pv", tag="psA")
                for sh in range(2):
                    for k in range(8):
                        nc.tensor.matmul(
                            pv[:, sh * 256 : sh * 256 + 256],
                            lhsT=ht_sb[:, k, ts(st0 + sh, 128)],
                            rhs=wv_sb[:, k, :],
                            start=(k == 0),
                            stop=(k == 7),
                        )
                nc.vector.tensor_copy(
                    out=v4r[:, st0 : st0 + 2, :, 0:64],
                    in_=pv[:, 0:512].rearrange("p (s g x) -> p s g x", s=2, x=64),
                )

            # ---------------- phase 1c: CUM (cumulative v-half sums) -------
            # cum[h][d, s] = sum_{t<=s} vlin_half[t, d]; row 64 = 0.5*(s+1)
            for h in range(2):
                lin_sl = slice(65 * h, 65 * h + 65)
                for sj in range(NT):
                    icp = psA.tile([128, 1024], f32, name="icp", tag="psA")
                    nc.tensor.matmul(
                        icp[0:65, 0:128],
                        lhsT=v4_sb[:, sj, lin_sl],
                        rhs=msk_sb[:, 0:128],
                        start=True,
                        stop=True,
                    )
                    if sj == 0:
                        nc.vector.tensor_scalar(
                            out=cum_sb[h][:, 0:128],
                            in0=icp[0:65, 0:128],
                            scalar1=0.0,
                            scalar2=None,
                            op0=ADD,
                        )
                    else:
                        nc.vector.tensor_scalar(
                            out=cum_sb[h][:, ts(sj, 128)],
                            in0=icp[0:65, 0:128],
                            scalar1=cum_sb[h][:, sj * 128 - 1 : sj * 128],
                            scalar2=None,
                            op0=ADD,
                        )

            # ---------------- phase 2 + pipelined tails ----------------
            def phase2(j):
                js = ts(j, SB)
                # ----- linear-attention chains, both heads per ti -----
                lim_l = 4 * j + 4
                qkv = [
                    psAcc.tile([65, SB], f32, name=f"qkv{h}", tag="acc")
                    for h in range(2)
                ]
                sq_tiles = {}

                def lin_acc(ti):
                    sqt = sq_tiles.pop(ti)
                    for h in range(2):
                        nc.tensor.matmul(
                            qkv[h][:, :],
                            lhsT=v4_sb[:, ti, slice(65 * h, 65 * h + 65)],
                            rhs=sqt[:, ts(h, SB)],
                            start=(ti == 0),
                            stop=(ti == lim_l - 1),
                            skip_group_check=True,
                        )

                for ti in range(lim_l):
                    mp = psA.tile([128, 1024], f32, name="mp", tag="psA")
                    for h, (p0, p1) in enumerate(((0, 17), (32, 49))):
                        nc.tensor.matmul(
                            mp[:, ts(h, SB)],
                            lhsT=qkg_sb[p0:p1, 1, ts(ti, 128)],
                            rhs=qkg_sb[p0:p1, 0, js],
                            start=True,
                            stop=True,
                        )
                    sq = sqp.tile([128, 1024], bf16, name="sq", tag="sq")
                    col0 = max(0, ti - 4 * j) * 128
                    sqr = sq.rearrange("p (g x) -> p g x", x=SB)
                    mpr = mp.rearrange("p (g x) -> p g x", x=SB)
                    if col0:
                        nc.gpsimd.memset(sqr[:, :, 0:col0], 0.0)
                    nc.scalar.activation(
                        sqr[:, :, col0:SB], mpr[:, :, col0:SB], Square
                    )
                    sd = ti - 4 * j
                    if 0 <= sd <= 3:
                        for h in range(2):
                            dsl = slice(h * SB + sd * 128, h * SB + (sd + 1) * 128)
                            nc.gpsimd.tensor_tensor(
                                out=sq[:, dsl], in0=sq[:, dsl],
                                in1=msk_sb[:, 0:128], op=MULT,
                            )
                    sq_tiles[ti] = sq
                    if ti >= 1:
                        lin_acc(ti - 1)
                lin_acc(lim_l - 1)
                for h in range(2):
                    nc.vector.tensor_tensor(
                        out=ul_sb[h][:, js],
                        in0=qkv[h][0:65, :],
                        in1=cum_sb[h][:, js],
                        op=ADD,
                    )
                    nc.sync.dma_start(
                        out=den_sb[32 * h : 32 * h + 1, js], in_=ul_sb[h][64:65, js]
                    )

                # ----- windowed-attention chains -----
                lim_w = min(16, 4 * j + 6)
                nt = [
                    psAcc.tile([65, SB], f32, name=f"nt{h}", tag="acc")
                    for h in range(2)
                ]
                ex_tiles = {}

                def win_acc(ti):
                    ext = ex_tiles.pop(ti)
                    for h in range(2):
                        nc.tensor.matmul(
                            nt[h][:, :],
                            lhsT=v4_sb[:, ti, slice(130 + 65 * h, 195 + 65 * h)],
                            rhs=ext[:, ts(h, SB)],
                            start=(ti == 0),
                            stop=(ti == lim_w - 1),
                            skip_group_check=True,
                        )

                for ti in range(lim_w):
                    sp = psA.tile([128, 1024], f32, name="sp", tag="psA")
                    for h in range(2):
                        hsl = slice(64 * h, 64 * h + 64)
                        nc.tensor.matmul(
                            sp[:, ts(h, SB)],
                            lhsT=kw_sb[hsl, ts(ti, 128)],
                            rhs=qw_sb[hsl, js],
                            start=True,
                            stop=True,
                        )
                    ex = exq.tile([128, 1024], bf16, name="ex", tag="ex")
                    col0 = max(0, ti - 2 - 4 * j) * 128
                    exr = ex.rearrange("p (g x) -> p g x", x=SB)
                    spr = sp.rearrange("p (g x) -> p g x", x=SB)
                    if col0:
                        nc.gpsimd.memset(exr[:, :, 0:col0], 0.0)
                    nc.scalar.activation(
                        exr[:, :, col0:SB], spr[:, :, col0:SB], Exp, scale=0.125
                    )
                    sd = ti - 2 - 4 * j
                    if 0 <= sd <= 3:
                        for h in range(2):
                            dsl = slice(h * SB + sd * 128, h * SB + (sd + 1) * 128)
                            nc.gpsimd.tensor_tensor(
                                out=ex[:, dsl], in0=ex[:, dsl],
                                in1=msk_sb[:, 128:256], op=MULT,
                            )
                    ex_tiles[ti] = ex
                    if ti >= 1:
                        win_acc(ti - 1)
                win_acc(lim_w - 1)
                for h in range(2):
                    nc.vector.tensor_copy(out=uw_sb[h][:, js], in_=nt[h][0:65, :])
                    nc.sync.dma_start(
                        out=den_sb[64 + 32 * h : 65 + 32 * h, js],
                        in_=uw_sb[h][64:65, js],
                    )

            def tail(j):
                js = ts(j, SB)
                nc.vector.tensor_scalar(
                    out=recf_sb[0:97, js], in0=den_sb[0:97, js],
                    scalar1=EPS, scalar2=None, op0=ADD,
                )
                nc.vector.reciprocal(out=recf_sb[0:97, js], in_=recf_sb[0:97, js])
                for r in range(4):
                    # partition_broadcast hardware reads physical partition 0,
                    # so stage each reciprocal row into a partition-0 tile
                    nc.sync.dma_start(
                        out=recr_sb[r][0:1, js],
                        in_=recf_sb[32 * r : 32 * r + 1, js],
                    )
                for h in range(2):
                    for p in range(2):  # 0 = lin, 1 = win
                        r = 2 * p + h
                        u = ul_sb[h] if p == 0 else uw_sb[h]
                        bc = bct.tile([64, SB], f32, name="bc", tag="bc")
                        nc.gpsimd.partition_broadcast(
                            bc[:, :], recr_sb[r][0:1, js]
                        )
                        nc.gpsimd.tensor_tensor(
                            out=scl_sb[h][ts(p, 64), js],
                            in0=u[0:64, js],
                            in1=bc[:, :],
                            op=MULT,
                        )
                for st in range(4 * j, 4 * j + 4):
                    po = psO.tile([128, 1024], f32, name="po", tag="po")
                    for nb in range(2):
                        for h in range(2):
                            nc.tensor.matmul(
                                po[:, ts(nb, SB)],
                                lhsT=scl_sb[h][:, ts(st, 128)],
                                rhs=wo_sb[:, h, ts(nb, SB)],
                                start=(h == 0),
                                stop=(h == 1),
                                skip_group_check=True,
                            )
                    so = stg.tile([128, 1024], f32, name="so", tag="so")
                    nc.vector.tensor_copy(out=so[:, :], in_=po[:, :])
                    nc.sync.dma_start(out=out_d[ts(st, 128), :], in_=so[:, :])

            phase2(0)
            phase2(1)
            tail(0)
            phase2(2)
            tail(1)
            phase2(3)
            tail(2)
            tail(3)
            if dbg:
                dmp = sb.tile([128, 4160], f32, name="dmp")
                def dump(name, t, rows, cols):
                    nc.vector.tensor_copy(out=dmp[0:rows, 0:cols], in_=t)
                    nc.sync.dma_start(
                        out=dbg_t[name][0:rows, 0:cols], in_=dmp[0:rows, 0:cols]
                    )
                dump("d_qw", qw_sb[:, :], 128, S)
                dump("d_kw", kw_sb[:, :], 128, S)
                dump("d_qkg", qkg_sb[0:49, :, :].rearrange("p a f -> p (a f)"), 49, 2 * S)
                dump("d_v4", v4_sb[:, :, :].rearrange("p a f -> p (a f)"), 128, 16 * 260)
                dump("d_cum0", cum_sb[0][:, :], 65, S)
                dump("d_ul0", ul_sb[0][:, :], 65, S)
                dump("d_uw0", uw_sb[0][:, :], 65, S)
                dump("d_den", den_sb[:, :], 128, S)
                dump("d_recf", recf_sb[0:97, :], 97, S)
                dump("d_scl0", scl_sb[0][:, :], 128, S)

    nc.compile()
    return nc


def _prep_inputs(inputs):
    """Host-side sharding/packing. Returns per-core input maps."""
    h = np.asarray(inputs["hidden_states"], np.float32).reshape(S, D)
    ht = np.ascontiguousarray(h.T).astype(BF)

    lin_Wq = np.asarray(inputs["lin_Wq"], np.float32)
    lin_Wk = np.asarray(inputs["lin_Wk"], np.float32)
    lin_Wv = np.asarray(inputs["lin_Wv"], np.float32)
    lin_Wo = np.asarray(inputs["lin_Wo"], np.float32)
    win_Wq = np.asarray(inputs["win_Wq"], np.float32)
    win_Wk = np.asarray(inputs["win_Wk"], np.float32)
    win_Wv = np.asarray(inputs["win_Wv"], np.float32)
    win_Wo = np.asarray(inputs["win_Wo"], np.float32)

    # constant mask tiles
    p = np.arange(128)[:, None]
    f = np.arange(128)[None, :]
    msk = np.zeros((128, 256), np.float32)
    msk[:, 0:128] = (p <= f)          # lin diag mask (t <= s)
    msk[:, 128:256] = (p < f)         # win partial mask (t < s)

    in_maps = []
    for c in range(NCORES):
        a, b = 2 * c, 2 * c + 1
        wqk = np.zeros((D, 384), np.float32)
        wqk[:, 0:64] = win_Wq[:, a * HD : (a + 1) * HD]
        wqk[:, 64:128] = win_Wq[:, b * HD : (b + 1) * HD]
        wqk[:, 128:192] = win_Wk[:, a * HD : (a + 1) * HD]
        wqk[:, 192:256] = win_Wk[:, b * HD : (b + 1) * HD]
        wqk[:, 256:272] = lin_Wq[:, a * FD : (a + 1) * FD] * 0.5
        wqk[:, 288:304] = lin_Wq[:, b * FD : (b + 1) * FD] * 0.5
        wqk[:, 320:336] = lin_Wk[:, a * FD : (a + 1) * FD] * 0.5
        wqk[:, 352:368] = lin_Wk[:, b * FD : (b + 1) * FD] * 0.5
        wv = np.zeros((D, 256), np.float32)
        wv[:, 0:64] = lin_Wv[:, a * HD : (a + 1) * HD] * 0.5
        wv[:, 64:128] = lin_Wv[:, b * HD : (b + 1) * HD] * 0.5
        wv[:, 128:192] = win_Wv[:, a * HD : (a + 1) * HD]
        wv[:, 192:256] = win_Wv[:, b * HD : (b + 1) * HD]
        wo = np.zeros((256, D), np.float32)
        wo[0:64] = lin_Wo[a * HD : (a + 1) * HD]
        wo[64:128] = win_Wo[a * HD : (a + 1) * HD]
        wo[128:192] = lin_Wo[b * HD : (b + 1) * HD]
        wo[192:256] = win_Wo[b * HD : (b + 1) * HD]
        in_maps.append(
            {
                "ht": ht,
                "wqk": wqk.astype(BF),
                "wv": wv.astype(BF),
                "wo": wo.astype(BF),
                "msk": msk.astype(BF),
                "orow": np.ones((1, S), np.float32).astype(BF),
            }
        )
    return in_maps


def kernel(**inputs) -> np.ndarray:
    from concourse.bass_utils import run_bass_kernel_spmd

    if "nc" not in _CACHE:
        _CACHE["nc"] = _build_nc()
    nc = _CACHE["nc"]
    in_maps = _prep_inputs(inputs)
    res = run_bass_kernel_spmd(nc, in_maps, core_ids=list(range(NCORES)))
    out = np.zeros((S, D), np.float32)
    for r in res.results:
        out += r["out"]
    return out.reshape(1, S, D)


if __name__ == "__main__":
    nc = _build_nc()
    print("built ok")
